# revision 26
# baseline (speedup 1.0000x reference)
"""Trainium2 Bass kernel for nn_LogicConv3d (differentiable logic-gate 3D conv).

Strategy
--------
The reference's big gather `x.reshape(B,-1)[:, lin]` reads shifted 30x30x30
windows of the (C,32,32,32) volume: coords lie in [0,3), so each (j,k,s) leaf
operand is one of 81 shifted slices (c,dh,dw,dd).  Each tree node is a
bilinear blend  out = c0 + ca*a + cb*b + cab*a*b  whose coefficients come from
softmax(w)@GATES — tiny, computed on host.  Constants are folded into parents
(the bilinear form is closed under constant shifts of its inputs).

Sharding: kernels K=32 split 4-per-core across 8 cores (batch packed into the
partition/flat-position dim).  Per-core differences are pure DATA, so ONE SPMD
program runs on all 8 cores via run_bass_kernel_spmd.

Device op mix (final): scalar_tensor_tensor has NO fast DVE mode (~1094ns
per (128,844) fp16 tile) while tensor_scalar runs ~494ns and tensor_tensor
~594ns (2x fp16 mode).  A per-node SCALE GAUGE eliminates STT entirely:
node (lev,i) emits o' = sigma*o with sigma = sigma_bchild/cb2 (clamped to
+-SIG_CAP for fp16 range; scale-only transforms are fp16-safe), making the
o-op a PURE add.  Per node:
    u = TS(b, s1, s2)         # ACT (scalar engine); s1,s2 host-folded
    t = TT_mult(a, u)         # DVE
    o' = TT_add(b, t)         # DVE (no scalars needed!)
The root uses u,t plus v = TS(b, cb2/sigB, gamma); out = TT_add(t, v) to
emit the exact value.  GPSIMD is unused: concurrent GPSIMD activity slows
DVE ops ~3.5x (net negative).  ACT and DVE both run ~138us/core, ~98%
packed via: eager post-order tree walk (minimal o-tile liveness),
2-kernel-interleaved streams, 3-stage software-pipelined emission
(O(j-2), TT(j-1), TS(j)), and sibling-leaf PAIRING (adjacent chunk slices
let leaf t/o ops fuse into (128, 2*844) tensor_tensor ops).

DMA: leaf operands are host-pre-gathered into per-kernel-chunk contiguous
arrays (4 leaves x 844 positions per chunk), arriving in 32 ~0.9MB DMAs on
the sync HWDGE ring; the first pair's b-chunks ride the ACT ring so the
startup a/b loads stream in parallel.  Outputs are fp16 (root constant
folded on-device), cast to fp32 on host.
"""
import numpy as np

# ---- problem constants (hardcoded per contest contract) ----
B, C, H, W, D = 4, 3, 32, 32, 32
K, S = 32, 16
OH = OW = OD = 30
P = OH * OW * OD            # 27000
BP = B * P                  # 108000
NPART = 128
FREE = (BP + NPART - 1) // NPART   # 844
PADBP = NPART * FREE        # 108032
NCORES = 8
KLOC = K // NCORES          # 4
TEMP = 1.0
NLEV = 5
NODES_PER_K = 31            # 16+8+4+2+1
CHUNK = 4                   # leaves per input DMA chunk
NCHUNK = S // CHUNK         # 4 per kernel per operand
CFREE = CHUNK * FREE        # 3376
NCOLS = KLOC * (30 * 2 + 4)  # 256 coef cols (30 non-root x2 [s1,s2] + root x4)
LEV_OFF = [0, 32, 48, 56, 60]  # per-kernel coef column offset by level
SIG_CAP = 8192.0            # scale-gauge clamp (keeps fp16 tiles in range)

GATES = np.array([[(g >> t) & 1 for t in range(4)] for g in range(16)],
                 dtype=np.float64)

# engine assignment knobs.  GPSIMD is net-negative (concurrent GPS activity
# slows DVE ops ~3.5x), so everything runs on DVE+ACT: all tensor_scalar
# (u/v) ops on ACT (scalar engine), all tensor_tensor (t-mult, o-add) on DVE.
# Scale-gauge: each non-root node emits o' = sigma*o with sigma chosen so
# the o-op is a PURE tensor add (o' = b_tile + t'), eliminating the slow
# scalar_tensor_tensor op; host folds all scales into the u-op scalars.
TS_DVE_RES = (5,)        # TS op -> DVE when ts_idx % TS_DVE_MODB in RES
TS_DVE_MODB = 18
USE_GPS = False


# ----------------------------------------------------------------- host math
def _lut_coeffs(w):
    """w: (nodes,K,16) -> c0, ca, cb, cab each (nodes,K) float64."""
    w = w.astype(np.float64)
    e = np.exp((w - w.max(-1, keepdims=True)) / TEMP)
    p = e / e.sum(-1, keepdims=True)
    l = p @ GATES
    l0, l1, l2, l3 = l[..., 0], l[..., 1], l[..., 2], l[..., 3]
    return l0, l2 - l0, l1 - l0, l0 - l1 - l2 + l3


def _fold_coeffs(ws):
    """Fold per-node constants into parents.  Returns (folded, root_const):
    folded[lev] = (ca2, cb2, cab) each (nodes,K); root_const (K,)."""
    folded = []
    gamma = None
    for lev, w in enumerate(ws):
        c0, ca, cb, cab = _lut_coeffs(w)
        if lev == 0:
            gA = np.zeros_like(c0)
            gB = np.zeros_like(c0)
        else:
            gA = gamma[0::2]
            gB = gamma[1::2]
        folded.append((ca + cab * gB, cb + cab * gA, cab))
        gamma = c0 + ca * gA + cb * gB + cab * gA * gB
    return folded, gamma[0]


def _coef_cols(k, folded, root_const):
    """Per-kernel coef column values, in (level, index) order.

    Scale-gauge: node (lev,i) emits o' = sigma*o.  sigma(leaf) = 1/cb2;
    sigma(lev,i) = sigma(lev-1, 2i+1)/cb2, clamped to +-SIG_CAP, so that
    o' = b_tile + t' is a pure add.  u-op scalars absorb everything:
    s1 = cab*sig/(sigA*sigB), s2 = ca2*sig/sigA.  Root emits the true value:
    s1 = cab/(sigA*sigB), s2 = ca2/sigA, v-op = (cb2/sigB)*b + root_const."""
    sig = {}
    cols = []
    for lev in range(NLEV - 1):
        ca2, cb2, cab = folded[lev]
        for i in range(ca2.shape[0]):
            if lev == 0:
                sA = sB = 1.0
            else:
                sA = sig[(lev - 1, 2 * i)]
                sB = sig[(lev - 1, 2 * i + 1)]
            sg = float(np.clip(sB / cb2[i, k], -SIG_CAP, SIG_CAP))
            sig[(lev, i)] = sg
            cols += [cab[i, k] * sg / (sA * sB), ca2[i, k] * sg / sA]
    ca2, cb2, cab = folded[NLEV - 1]
    sA = sig[(NLEV - 2, 0)]
    sB = sig[(NLEV - 2, 1)]
    cols += [cab[0, k] / (sA * sB), ca2[0, k] / sA,
             cb2[0, k] / sB, root_const[k]]
    return cols


def _prep_inputs(x, kc, ws):
    """Build per-core in_maps (numpy)."""
    # 81 shifted windows, flattened positions (b,oh,ow,od), fp16, padded
    X81 = np.empty((3, 3, 3, 3, B, OH, OW, OD), np.float32)
    for c in range(3):
        for dh in range(3):
            for dw in range(3):
                for dd in range(3):
                    X81[c, dh, dw, dd] = x[:, c, dh:dh + 30, dw:dw + 30, dd:dd + 30]
    X81f = np.zeros((81, NPART, FREE), np.float16)
    X81f.reshape(81, PADBP)[:, :BP] = X81.reshape(81, BP).astype(np.float16)

    h_, w_, d_, c_ = kc[..., 0], kc[..., 1], kc[..., 2], kc[..., 3]
    sl = ((c_ * 3 + h_) * 3 + w_) * 3 + d_          # (2,K,S)

    folded, root_const = _fold_coeffs(ws)

    in_maps = []
    for core in range(NCORES):
        ks = range(core * KLOC, (core + 1) * KLOC)
        a_in = np.empty((KLOC * NCHUNK, NPART, CFREE), np.float16)
        b_in = np.empty((KLOC * NCHUNK, NPART, CFREE), np.float16)
        colv = []
        for kk, k in enumerate(ks):
            for c in range(NCHUNK):
                # in-chunk leaf order [4c, 4c+2, 4c+1, 4c+3]: the lev0 quad
                # output tile then holds lev1's a-inputs in its left half and
                # b-inputs in its right half (enables fused quad/pair ops)
                perm = 4 * c + np.array([0, 2, 1, 3])
                idx0 = sl[0, k, perm]
                idx1 = sl[1, k, perm]
                a_in[kk * NCHUNK + c] = \
                    X81f[idx0].transpose(1, 0, 2).reshape(NPART, CFREE)
                b_in[kk * NCHUNK + c] = \
                    X81f[idx1].transpose(1, 0, 2).reshape(NPART, CFREE)
            colv += _coef_cols(k, folded, root_const)
        assert len(colv) == NCOLS
        coef = np.broadcast_to(
            np.asarray(colv, np.float32), (NPART, NCOLS)).copy()
        in_maps.append({"a_in": a_in, "b_in": b_in, "coef": coef})
    return in_maps


# ------------------------------------------------------------ device program
def _build_program():
    import concourse.bass as bass
    import concourse.bacc as bacc
    import concourse.mybir as mybir
    from concourse.tile import TileContext

    f16 = mybir.dt.float16
    f32 = mybir.dt.float32
    Alu = mybir.AluOpType
    Act = mybir.ActivationFunctionType

    nc = bacc.Bacc()
    a_in = nc.declare_dram_parameter("a_in", [KLOC * NCHUNK, NPART, CFREE],
                                     f16, isOutput=False)
    b_in = nc.declare_dram_parameter("b_in", [KLOC * NCHUNK, NPART, CFREE],
                                     f16, isOutput=False)
    coef = nc.declare_dram_parameter("coef", [NPART, NCOLS], f32,
                                     isOutput=False)
    out = nc.declare_dram_parameter("out", [KLOC, NPART, FREE], f16,
                                    isOutput=True)

    ts_idx = 0
    o_idx = 0

    def eager_nodes():
        """Eager node sequence for one kernel.  ('Q', q) = level-0 QUAD
        (chunk q, 4 leaves, fused (128,4*FREE) t/o ops); ('P', q) = level-1
        PAIR (nodes 2q, 2q+1, fused (128,2*FREE) ops); (lev, i) = single
        node at levels 2+.  Interleaving two kernels doubles every
        producer-consumer stream distance, keeping the 3-stage pipeline
        lag satisfied."""
        return [("Q", 0), ("Q", 1), ("P", 0), ("P", 1), (2, 0), (2, 1),
                ("Q", 2), (3, 0), ("Q", 3), ("P", 2), ("P", 3), (2, 2),
                (2, 3), (3, 1), (4, 0)]

    with TileContext(nc) as tc:
        with (
            tc.tile_pool(name="cpool", bufs=1) as cpool,
            tc.tile_pool(name="apool", bufs=6) as apool,
            tc.tile_pool(name="bpool", bufs=6) as bpool,
            tc.tile_pool(name="upool", bufs=5) as upool,
            tc.tile_pool(name="vpool", bufs=2) as vpool,
            tc.tile_pool(name="tpool", bufs=5) as tpool,
            tc.tile_pool(name="lpool", bufs=2) as lpool,
            tc.tile_pool(name="opool", bufs=3) as opool,
        ):
            coef_sb = cpool.tile([NPART, NCOLS], f32)
            nc.sync.dma_start(out=coef_sb[:], in_=coef[:])

            def ts_op(dst, src, scale_ap, bias_ap):
                nonlocal ts_idx
                if ts_idx % TS_DVE_MODB in TS_DVE_RES:
                    if bias_ap is None:
                        nc.vector.tensor_scalar(dst, src, scale_ap, None,
                                                Alu.mult)
                    else:
                        nc.vector.tensor_scalar(dst, src, scale_ap, bias_ap,
                                                Alu.mult, Alu.add)
                else:
                    nc.scalar.activation(dst, src, Act.Identity,
                                         bias=bias_ap if bias_ap is not None
                                         else 0.0,
                                         scale=scale_ap)
                ts_idx += 1

            # per-(kernel, lev, idx) state
            achunk = {}
            bchunk = {}
            otile = {}
            state = {}

            def col_of(kk, lev, i):
                return kk * 64 + LEV_OFF[lev] + (4 if lev == NLEV - 1
                                                 else 2) * i

            QPERM = (0, 2, 1, 3)

            def inputs(kk, lev, i):
                if lev == 2:
                    pr = otile[kk, "P", i]
                    return pr[:, :FREE], pr[:, FREE:]
                return (otile[kk, lev - 1, 2 * i][:],
                        otile[kk, lev - 1, 2 * i + 1][:])

            def stage_ts(kk, lev, i):
                if lev == "Q":
                    u4 = upool.tile([NPART, 4 * FREE], f16, tag="u4",
                                    name=f"u4_{kk}_{i}", bufs=2)
                    for h in range(4):
                        leaf = 4 * i + QPERM[h]
                        col = col_of(kk, 0, leaf)
                        bh = bchunk[kk, i][:, h * FREE:(h + 1) * FREE]
                        ts_op(u4[:, h * FREE:(h + 1) * FREE], bh,
                              coef_sb[:, col:col + 1],
                              coef_sb[:, col + 1:col + 2])
                    state[kk, lev, i] = (u4, None)
                    return
                if lev == "P":
                    oq = otile[kk, "Q", i]
                    u2 = upool.tile([NPART, 2 * FREE], f16, tag="u2",
                                    name=f"u2_{kk}_{i}", bufs=3)
                    for h in (0, 1):
                        col = col_of(kk, 1, 2 * i + h)
                        bh = oq[:, (2 + h) * FREE:(3 + h) * FREE]
                        ts_op(u2[:, h * FREE:(h + 1) * FREE], bh,
                              coef_sb[:, col:col + 1],
                              coef_sb[:, col + 1:col + 2])
                    state[kk, lev, i] = (u2, None)
                    return
                col = col_of(kk, lev, i)
                a_ap, b_ap = inputs(kk, lev, i)
                is_root = lev == NLEV - 1
                u = upool.tile([NPART, FREE], f16, tag="u",
                               name=f"u{kk}_{lev}_{i}", bufs=3)
                ts_op(u[:], b_ap, coef_sb[:, col:col + 1],
                      coef_sb[:, col + 1:col + 2])
                v = None
                if is_root:
                    v = vpool.tile([NPART, FREE], f16, tag="v",
                                   name=f"v{kk}_{lev}_{i}")
                    ts_op(v[:], b_ap, coef_sb[:, col + 2:col + 3],
                          coef_sb[:, col + 3:col + 4])
                state[kk, lev, i] = (u, v, a_ap, b_ap)

            def stage_tt(kk, lev, i):
                if lev == "Q":
                    u4, _ = state[kk, lev, i]
                    t4 = tpool.tile([NPART, 4 * FREE], f16, tag="t4",
                                    name=f"t4_{kk}_{i}", bufs=2)
                    nc.vector.tensor_tensor(out=t4[:], in0=achunk[kk, i][:],
                                            in1=u4[:], op=Alu.mult)
                    state[kk, lev, i] = (t4, None)
                    return
                if lev == "P":
                    u2, _ = state[kk, lev, i]
                    oq = otile[kk, "Q", i]
                    t2 = tpool.tile([NPART, 2 * FREE], f16, tag="t2",
                                    name=f"t2_{kk}_{i}", bufs=3)
                    nc.vector.tensor_tensor(out=t2[:],
                                            in0=oq[:, :2 * FREE],
                                            in1=u2[:], op=Alu.mult)
                    state[kk, lev, i] = (t2, None)
                    return
                u, v, a_ap, b_ap = state[kk, lev, i]
                t = tpool.tile([NPART, FREE], f16, tag="t",
                               name=f"t{kk}_{lev}_{i}", bufs=3)
                nc.vector.tensor_tensor(out=t[:], in0=a_ap, in1=u[:],
                                        op=Alu.mult)
                state[kk, lev, i] = (t, v, a_ap, b_ap)

            def stage_o(kk, lev, i):
                if lev == "Q":
                    t4, _ = state.pop((kk, lev, i))
                    o4 = lpool.tile([NPART, 4 * FREE], f16, tag="oq",
                                    name=f"o4_{kk}_{i}", bufs=4)
                    nc.vector.tensor_tensor(out=o4[:], in0=bchunk[kk, i][:],
                                            in1=t4[:], op=Alu.add)
                    otile[kk, "Q", i] = o4
                    return
                if lev == "P":
                    t2, _ = state.pop((kk, lev, i))
                    oq = otile[kk, "Q", i]
                    o2 = lpool.tile([NPART, 2 * FREE], f16, tag="o1p",
                                    name=f"o2_{kk}_{i}", bufs=3)
                    nc.vector.tensor_tensor(out=o2[:],
                                            in0=oq[:, 2 * FREE:],
                                            in1=t2[:], op=Alu.add)
                    otile[kk, "P", i] = o2
                    return
                t, v, a_ap, b_ap = state.pop((kk, lev, i))
                is_root = lev == NLEV - 1
                if is_root:
                    ot = opool.tile([NPART, FREE], f16, tag="out",
                                    name=f"ot{kk}")
                    nc.vector.tensor_tensor(out=ot[:], in0=t[:], in1=v[:],
                                            op=Alu.add)
                    nc.sync.dma_start(out=out[kk], in_=ot[:])
                    return
                o = lpool.tile([NPART, FREE], f16, tag=f"o{lev}",
                               name=f"o{kk}_{lev}_{i}", bufs=3)
                nc.vector.tensor_tensor(out=o[:], in0=b_ap, in1=t[:],
                                        op=Alu.add)
                otile[kk, lev, i] = o

            for pair in range(KLOC // 2):
                kA, kB = 2 * pair, 2 * pair + 1
                for c in range(NCHUNK):
                    for kk in (kA, kB):
                        at = apool.tile([NPART, CFREE], f16, tag="a",
                                        name=f"a{kk}_{c}")
                        nc.sync.dma_start(out=at[:],
                                          in_=a_in[kk * NCHUNK + c])
                        achunk[kk, c] = at
                        bt = bpool.tile([NPART, CFREE], f16, tag="b",
                                        name=f"b{kk}_{c}")
                        # first pair's chunk-0 b-loads ride the ACT HWDGE
                        # ring, QUARTERED so the first u-ops can start after
                        # ~1.3us of data; the rest stay whole on sync (ACT
                        # issue time is costly)
                        if pair == 0 and c == 0:
                            src = b_in[kk * NCHUNK + c]
                            for h in range(4):
                                sl_ = slice(h * FREE, (h + 1) * FREE)
                                nc.scalar.dma_start(out=bt[:, sl_],
                                                    in_=src[:, sl_])
                        else:
                            nc.sync.dma_start(out=bt[:],
                                              in_=b_in[kk * NCHUNK + c])
                        bchunk[kk, c] = bt
                # interleave the two kernels' eager node streams
                nodes = []
                for na, nb in zip(eager_nodes(), eager_nodes()):
                    nodes.append((kA,) + na)
                    nodes.append((kB,) + nb)
                # software-pipelined emission: O(j-2), TT(j-1), TS(j)
                n = len(nodes)
                for j in range(n + 2):
                    if j >= 2:
                        stage_o(*nodes[j - 2])
                    if 1 <= j <= n:
                        stage_tt(*nodes[j - 1])
                    if j < n:
                        stage_ts(*nodes[j])
    nc.compile()
    return nc


_PROGRAM = None


def kernel(**inputs):
    global _PROGRAM
    x = np.asarray(inputs["x"], dtype=np.float32)
    kc = np.asarray(inputs["kernel_coords"])
    ws = [np.asarray(inputs[f"w{i}"]) for i in range(5)]

    in_maps = _prep_inputs(x, kc, ws)

    from concourse.bass_utils import run_bass_kernel_spmd
    if _PROGRAM is None:
        _PROGRAM = _build_program()
    res = run_bass_kernel_spmd(_PROGRAM, in_maps, list(range(NCORES)))
    results = res.results

    full = np.empty((K, PADBP), np.float32)
    for core in range(NCORES):
        o = results[core]["out"].reshape(KLOC, PADBP)
        full[core * KLOC:(core + 1) * KLOC] = o
    out = full[:, :BP].reshape(K, B, OH, OW, OD).transpose(1, 0, 2, 3, 4)
    return np.ascontiguousarray(out)


# revision 28
# speedup vs baseline: 1.0198x; 1.0198x over previous
"""Trainium2 Bass kernel for nn_LogicConv3d (differentiable logic-gate 3D conv).

Strategy
--------
The reference's big gather `x.reshape(B,-1)[:, lin]` reads shifted 30x30x30
windows of the (C,32,32,32) volume: coords lie in [0,3), so each (j,k,s) leaf
operand is one of 81 shifted slices (c,dh,dw,dd).  Each tree node is a
bilinear blend  out = c0 + ca*a + cb*b + cab*a*b  whose coefficients come from
softmax(w)@GATES — tiny, computed on host.  Constants are folded into parents
(the bilinear form is closed under constant shifts of its inputs).

Sharding: kernels K=32 split 4-per-core across 8 cores (batch packed into the
partition/flat-position dim).  Per-core differences are pure DATA, so ONE SPMD
program runs on all 8 cores via run_bass_kernel_spmd.

Device op mix (final): scalar_tensor_tensor has NO fast DVE mode (~1094ns
per (128,844) fp16 tile) while tensor_scalar runs ~494ns and tensor_tensor
~594ns (2x fp16 mode).  A per-node SCALE GAUGE eliminates STT entirely:
node (lev,i) emits o' = sigma*o with sigma = sigma_bchild/cb2 (clamped to
+-SIG_CAP for fp16 range; scale-only transforms are fp16-safe), making the
o-op a PURE add.  Per node:
    u = TS(b, s1, s2)         # ACT (scalar engine); s1,s2 host-folded
    t = TT_mult(a, u)         # DVE
    o' = TT_add(b, t)         # DVE (no scalars needed!)
The root uses u,t plus v = TS(b, cb2/sigB, gamma); out = TT_add(t, v) to
emit the exact value.  GPSIMD is unused: concurrent GPSIMD activity slows
DVE ops ~3.5x (net negative).  ACT and DVE both run ~138us/core, ~98%
packed via: eager tree walk (minimal o-tile liveness), 2-kernel-interleaved
streams, 3-stage software-pipelined emission (O(j-2), TT(j-1), TS(j)), and
FUSED WIDE OPS: each 4-leaf chunk is host-ordered [4q, 4q+2, 4q+1, 4q+3]
so level 0 runs as one (128,4*844) t-op + o-add per chunk whose output
quad holds level-1's a-inputs in its left half and b-inputs in its right
half; level 1 then runs as (128,2*844) pair ops whose outputs are exactly
level-2's (a,b) halves.  ~7 tensor_scalar ops shift to DVE to rebalance
(TS_DVE_RES knob).

DMA: leaf operands are host-pre-gathered into per-kernel-chunk contiguous
arrays (4 leaves x 844 positions per chunk), arriving in 32 ~0.9MB DMAs on
the sync HWDGE ring; the first pair's b-chunks ride the ACT ring so the
startup a/b loads stream in parallel.  Outputs are fp16 (root constant
folded on-device), cast to fp32 on host.
"""
import numpy as np

# ---- problem constants (hardcoded per contest contract) ----
B, C, H, W, D = 4, 3, 32, 32, 32
K, S = 32, 16
OH = OW = OD = 30
P = OH * OW * OD            # 27000
BP = B * P                  # 108000
NPART = 128
FREE = (BP + NPART - 1) // NPART   # 844
PADBP = NPART * FREE        # 108032
NCORES = 8
KLOC = K // NCORES          # 4
TEMP = 1.0
NLEV = 5
NODES_PER_K = 31            # 16+8+4+2+1
CHUNK = 4                   # leaves per input DMA chunk
NCHUNK = S // CHUNK         # 4 per kernel per operand
CFREE = CHUNK * FREE        # 3376
NCOLS = KLOC * (30 * 2 + 4)  # 256 coef cols (30 non-root x2 [s1,s2] + root x4)
LEV_OFF = [0, 32, 48, 56, 60]  # per-kernel coef column offset by level
SIG_CAP = 8192.0            # scale-gauge clamp (keeps fp16 tiles in range)

GATES = np.array([[(g >> t) & 1 for t in range(4)] for g in range(16)],
                 dtype=np.float64)

# engine assignment knobs.  GPSIMD is net-negative (concurrent GPS activity
# slows DVE ops ~3.5x), so everything runs on DVE+ACT: all tensor_scalar
# (u/v) ops on ACT (scalar engine), all tensor_tensor (t-mult, o-add) on DVE.
# Scale-gauge: each non-root node emits o' = sigma*o with sigma chosen so
# the o-op is a PURE tensor add (o' = b_tile + t'), eliminating the slow
# scalar_tensor_tensor op; host folds all scales into the u-op scalars.
TS_DVE_RES = (5,)        # TS op -> DVE when ts_idx % TS_DVE_MODB in RES
TS_DVE_MODB = 18
USE_GPS = False


# ----------------------------------------------------------------- host math
def _lut_coeffs(w):
    """w: (nodes,K,16) -> c0, ca, cb, cab each (nodes,K) float64."""
    w = w.astype(np.float64)
    e = np.exp((w - w.max(-1, keepdims=True)) / TEMP)
    p = e / e.sum(-1, keepdims=True)
    l = p @ GATES
    l0, l1, l2, l3 = l[..., 0], l[..., 1], l[..., 2], l[..., 3]
    return l0, l2 - l0, l1 - l0, l0 - l1 - l2 + l3


def _fold_coeffs(ws):
    """Fold per-node constants into parents.  Returns (folded, root_const):
    folded[lev] = (ca2, cb2, cab) each (nodes,K); root_const (K,)."""
    folded = []
    gamma = None
    for lev, w in enumerate(ws):
        c0, ca, cb, cab = _lut_coeffs(w)
        if lev == 0:
            gA = np.zeros_like(c0)
            gB = np.zeros_like(c0)
        else:
            gA = gamma[0::2]
            gB = gamma[1::2]
        folded.append((ca + cab * gB, cb + cab * gA, cab))
        gamma = c0 + ca * gA + cb * gB + cab * gA * gB
    return folded, gamma[0]


def _coef_cols(k, folded, root_const):
    """Per-kernel coef column values, in (level, index) order.

    Scale-gauge: node (lev,i) emits o' = sigma*o.  sigma(leaf) = 1/cb2;
    sigma(lev,i) = sigma(lev-1, 2i+1)/cb2, clamped to +-SIG_CAP, so that
    o' = b_tile + t' is a pure add.  u-op scalars absorb everything:
    s1 = cab*sig/(sigA*sigB), s2 = ca2*sig/sigA.  Root emits the true value:
    s1 = cab/(sigA*sigB), s2 = ca2/sigA, v-op = (cb2/sigB)*b + root_const."""
    sig = {}
    cols = []
    for lev in range(NLEV - 1):
        ca2, cb2, cab = folded[lev]
        for i in range(ca2.shape[0]):
            if lev == 0:
                sA = sB = 1.0
            else:
                sA = sig[(lev - 1, 2 * i)]
                sB = sig[(lev - 1, 2 * i + 1)]
            sg = float(np.clip(sB / cb2[i, k], -SIG_CAP, SIG_CAP))
            sig[(lev, i)] = sg
            cols += [cab[i, k] * sg / (sA * sB), ca2[i, k] * sg / sA]
    ca2, cb2, cab = folded[NLEV - 1]
    sA = sig[(NLEV - 2, 0)]
    sB = sig[(NLEV - 2, 1)]
    cols += [cab[0, k] / (sA * sB), ca2[0, k] / sA,
             cb2[0, k] / sB, root_const[k]]
    return cols


def _prep_inputs(x, kc, ws):
    """Build per-core in_maps (numpy)."""
    # 81 shifted windows, flattened positions (b,oh,ow,od), fp16, padded
    X81 = np.empty((3, 3, 3, 3, B, OH, OW, OD), np.float32)
    for c in range(3):
        for dh in range(3):
            for dw in range(3):
                for dd in range(3):
                    X81[c, dh, dw, dd] = x[:, c, dh:dh + 30, dw:dw + 30, dd:dd + 30]
    X81f = np.zeros((81, NPART, FREE), np.float16)
    X81f.reshape(81, PADBP)[:, :BP] = X81.reshape(81, BP).astype(np.float16)

    h_, w_, d_, c_ = kc[..., 0], kc[..., 1], kc[..., 2], kc[..., 3]
    sl = ((c_ * 3 + h_) * 3 + w_) * 3 + d_          # (2,K,S)

    folded, root_const = _fold_coeffs(ws)

    in_maps = []
    for core in range(NCORES):
        ks = range(core * KLOC, (core + 1) * KLOC)
        a_in = np.empty((KLOC * NCHUNK, NPART, CFREE), np.float16)
        b_in = np.empty((KLOC * NCHUNK, NPART, CFREE), np.float16)
        colv = []
        for kk, k in enumerate(ks):
            for c in range(NCHUNK):
                # in-chunk leaf order [4c, 4c+2, 4c+1, 4c+3]: the lev0 quad
                # output tile then holds lev1's a-inputs in its left half and
                # b-inputs in its right half (enables fused quad/pair ops)
                perm = 4 * c + np.array([0, 2, 1, 3])
                idx0 = sl[0, k, perm]
                idx1 = sl[1, k, perm]
                a_in[kk * NCHUNK + c] = \
                    X81f[idx0].transpose(1, 0, 2).reshape(NPART, CFREE)
                b_in[kk * NCHUNK + c] = \
                    X81f[idx1].transpose(1, 0, 2).reshape(NPART, CFREE)
            colv += _coef_cols(k, folded, root_const)
        assert len(colv) == NCOLS
        coef = np.broadcast_to(
            np.asarray(colv, np.float32), (NPART, NCOLS)).copy()
        in_maps.append({"a_in": a_in, "b_in": b_in, "coef": coef})
    return in_maps


# ------------------------------------------------------------ device program
def _build_program():
    import concourse.bass as bass
    import concourse.bacc as bacc
    import concourse.mybir as mybir
    from concourse.tile import TileContext

    f16 = mybir.dt.float16
    f32 = mybir.dt.float32
    Alu = mybir.AluOpType
    Act = mybir.ActivationFunctionType

    nc = bacc.Bacc()
    a_in = nc.declare_dram_parameter("a_in", [KLOC * NCHUNK, NPART, CFREE],
                                     f16, isOutput=False)
    b_in = nc.declare_dram_parameter("b_in", [KLOC * NCHUNK, NPART, CFREE],
                                     f16, isOutput=False)
    coef = nc.declare_dram_parameter("coef", [NPART, NCOLS], f32,
                                     isOutput=False)
    out = nc.declare_dram_parameter("out", [KLOC, NPART, FREE], f16,
                                    isOutput=True)

    ts_idx = 0
    o_idx = 0

    def eager_nodes():
        """Eager node sequence for one kernel.  ('Q', q) = level-0 QUAD
        (chunk q, 4 leaves, fused (128,4*FREE) t/o ops); ('P', q) = level-1
        PAIR (nodes 2q, 2q+1, fused (128,2*FREE) ops); (lev, i) = single
        node at levels 2+.  Interleaving two kernels doubles every
        producer-consumer stream distance, keeping the 3-stage pipeline
        lag satisfied."""
        return [("Q", 0), ("Q", 1), ("P", 0), ("P", 1), (2, 0), (2, 1),
                ("Q", 2), (3, 0), ("Q", 3), ("P", 2), ("P", 3), (2, 2),
                (2, 3), (3, 1), (4, 0)]

    with TileContext(nc) as tc:
        with (
            tc.tile_pool(name="cpool", bufs=1) as cpool,
            tc.tile_pool(name="apool", bufs=6) as apool,
            tc.tile_pool(name="bpool", bufs=6) as bpool,
            tc.tile_pool(name="upool", bufs=5) as upool,
            tc.tile_pool(name="vpool", bufs=2) as vpool,
            tc.tile_pool(name="tpool", bufs=5) as tpool,
            tc.tile_pool(name="lpool", bufs=2) as lpool,
            tc.tile_pool(name="opool", bufs=3) as opool,
        ):
            coef_sb = cpool.tile([NPART, NCOLS], f32)
            nc.sync.dma_start(out=coef_sb[:], in_=coef[:])

            def ts_op(dst, src, scale_ap, bias_ap):
                nonlocal ts_idx
                if ts_idx % TS_DVE_MODB in TS_DVE_RES:
                    if bias_ap is None:
                        nc.vector.tensor_scalar(dst, src, scale_ap, None,
                                                Alu.mult)
                    else:
                        nc.vector.tensor_scalar(dst, src, scale_ap, bias_ap,
                                                Alu.mult, Alu.add)
                else:
                    nc.scalar.activation(dst, src, Act.Identity,
                                         bias=bias_ap if bias_ap is not None
                                         else 0.0,
                                         scale=scale_ap)
                ts_idx += 1

            # per-(kernel, lev, idx) state
            achunk = {}
            bchunk = {}
            otile = {}
            state = {}

            def col_of(kk, lev, i):
                return kk * 64 + LEV_OFF[lev] + (4 if lev == NLEV - 1
                                                 else 2) * i

            QPERM = (0, 2, 1, 3)

            def inputs(kk, lev, i):
                if lev == 2:
                    pr = otile[kk, "P", i]
                    return pr[:, :FREE], pr[:, FREE:]
                return (otile[kk, lev - 1, 2 * i][:],
                        otile[kk, lev - 1, 2 * i + 1][:])

            def stage_ts(kk, lev, i):
                if lev == "Q":
                    u4 = upool.tile([NPART, 4 * FREE], f16, tag="u4",
                                    name=f"u4_{kk}_{i}", bufs=2)
                    for h in range(4):
                        leaf = 4 * i + QPERM[h]
                        col = col_of(kk, 0, leaf)
                        bh = bchunk[kk, i][:, h * FREE:(h + 1) * FREE]
                        ts_op(u4[:, h * FREE:(h + 1) * FREE], bh,
                              coef_sb[:, col:col + 1],
                              coef_sb[:, col + 1:col + 2])
                    state[kk, lev, i] = (u4, None)
                    return
                if lev == "P":
                    oq = otile[kk, "Q", i]
                    u2 = upool.tile([NPART, 2 * FREE], f16, tag="u2",
                                    name=f"u2_{kk}_{i}", bufs=3)
                    for h in (0, 1):
                        col = col_of(kk, 1, 2 * i + h)
                        bh = oq[:, (2 + h) * FREE:(3 + h) * FREE]
                        ts_op(u2[:, h * FREE:(h + 1) * FREE], bh,
                              coef_sb[:, col:col + 1],
                              coef_sb[:, col + 1:col + 2])
                    state[kk, lev, i] = (u2, None)
                    return
                col = col_of(kk, lev, i)
                a_ap, b_ap = inputs(kk, lev, i)
                is_root = lev == NLEV - 1
                u = upool.tile([NPART, FREE], f16, tag="u",
                               name=f"u{kk}_{lev}_{i}", bufs=3)
                ts_op(u[:], b_ap, coef_sb[:, col:col + 1],
                      coef_sb[:, col + 1:col + 2])
                v = None
                if is_root:
                    v = vpool.tile([NPART, FREE], f16, tag="v",
                                   name=f"v{kk}_{lev}_{i}")
                    ts_op(v[:], b_ap, coef_sb[:, col + 2:col + 3],
                          coef_sb[:, col + 3:col + 4])
                state[kk, lev, i] = (u, v, a_ap, b_ap)

            def stage_tt(kk, lev, i):
                if lev == "Q":
                    u4, _ = state[kk, lev, i]
                    t4 = tpool.tile([NPART, 4 * FREE], f16, tag="t4",
                                    name=f"t4_{kk}_{i}", bufs=2)
                    nc.vector.tensor_tensor(out=t4[:], in0=achunk[kk, i][:],
                                            in1=u4[:], op=Alu.mult)
                    state[kk, lev, i] = (t4, None)
                    return
                if lev == "P":
                    u2, _ = state[kk, lev, i]
                    oq = otile[kk, "Q", i]
                    t2 = tpool.tile([NPART, 2 * FREE], f16, tag="t2",
                                    name=f"t2_{kk}_{i}", bufs=3)
                    nc.vector.tensor_tensor(out=t2[:],
                                            in0=oq[:, :2 * FREE],
                                            in1=u2[:], op=Alu.mult)
                    state[kk, lev, i] = (t2, None)
                    return
                u, v, a_ap, b_ap = state[kk, lev, i]
                t = tpool.tile([NPART, FREE], f16, tag="t",
                               name=f"t{kk}_{lev}_{i}", bufs=3)
                nc.vector.tensor_tensor(out=t[:], in0=a_ap, in1=u[:],
                                        op=Alu.mult)
                state[kk, lev, i] = (t, v, a_ap, b_ap)

            def stage_o(kk, lev, i):
                if lev == "Q":
                    t4, _ = state.pop((kk, lev, i))
                    o4 = lpool.tile([NPART, 4 * FREE], f16, tag="oq",
                                    name=f"o4_{kk}_{i}", bufs=4)
                    nc.vector.tensor_tensor(out=o4[:], in0=bchunk[kk, i][:],
                                            in1=t4[:], op=Alu.add)
                    otile[kk, "Q", i] = o4
                    return
                if lev == "P":
                    t2, _ = state.pop((kk, lev, i))
                    oq = otile[kk, "Q", i]
                    o2 = lpool.tile([NPART, 2 * FREE], f16, tag="o1p",
                                    name=f"o2_{kk}_{i}", bufs=3)
                    nc.vector.tensor_tensor(out=o2[:],
                                            in0=oq[:, 2 * FREE:],
                                            in1=t2[:], op=Alu.add)
                    otile[kk, "P", i] = o2
                    return
                t, v, a_ap, b_ap = state.pop((kk, lev, i))
                is_root = lev == NLEV - 1
                if is_root:
                    ot = opool.tile([NPART, FREE], f16, tag="out",
                                    name=f"ot{kk}")
                    nc.vector.tensor_tensor(out=ot[:], in0=t[:], in1=v[:],
                                            op=Alu.add)
                    nc.sync.dma_start(out=out[kk], in_=ot[:])
                    return
                o = lpool.tile([NPART, FREE], f16, tag=f"o{lev}",
                               name=f"o{kk}_{lev}_{i}", bufs=3)
                nc.vector.tensor_tensor(out=o[:], in0=b_ap, in1=t[:],
                                        op=Alu.add)
                otile[kk, lev, i] = o

            for pair in range(KLOC // 2):
                kA, kB = 2 * pair, 2 * pair + 1
                for c in range(NCHUNK):
                    for kk in (kA, kB):
                        at = apool.tile([NPART, CFREE], f16, tag="a",
                                        name=f"a{kk}_{c}")
                        nc.sync.dma_start(out=at[:],
                                          in_=a_in[kk * NCHUNK + c])
                        achunk[kk, c] = at
                        bt = bpool.tile([NPART, CFREE], f16, tag="b",
                                        name=f"b{kk}_{c}")
                        # first pair's chunk-0 b-loads ride the ACT HWDGE
                        # ring so the startup a/b DMAs stream in parallel;
                        # the rest stay whole on sync (ACT issue is costly)
                        if pair == 0 and c == 0:
                            nc.scalar.dma_start(out=bt[:],
                                                in_=b_in[kk * NCHUNK + c])
                        else:
                            nc.sync.dma_start(out=bt[:],
                                              in_=b_in[kk * NCHUNK + c])
                        bchunk[kk, c] = bt
                # interleave the two kernels' eager node streams
                nodes = []
                for na, nb in zip(eager_nodes(), eager_nodes()):
                    nodes.append((kA,) + na)
                    nodes.append((kB,) + nb)
                # software-pipelined emission: O(j-2), TT(j-1), TS(j)
                n = len(nodes)
                for j in range(n + 2):
                    if j >= 2:
                        stage_o(*nodes[j - 2])
                    if 1 <= j <= n:
                        stage_tt(*nodes[j - 1])
                    if j < n:
                        stage_ts(*nodes[j])
    nc.compile()
    return nc


_PROGRAM = None


def kernel(**inputs):
    global _PROGRAM
    x = np.asarray(inputs["x"], dtype=np.float32)
    kc = np.asarray(inputs["kernel_coords"])
    ws = [np.asarray(inputs[f"w{i}"]) for i in range(5)]

    in_maps = _prep_inputs(x, kc, ws)

    from concourse.bass_utils import run_bass_kernel_spmd
    if _PROGRAM is None:
        _PROGRAM = _build_program()
    res = run_bass_kernel_spmd(_PROGRAM, in_maps, list(range(NCORES)))
    results = res.results

    full = np.empty((K, PADBP), np.float32)
    for core in range(NCORES):
        o = results[core]["out"].reshape(KLOC, PADBP)
        full[core * KLOC:(core + 1) * KLOC] = o
    out = full[:, :BP].reshape(K, B, OH, OW, OD).transpose(1, 0, 2, 3, 4)
    return np.ascontiguousarray(out)
